# revision 1
# baseline (speedup 1.0000x reference)
"""Multi-head attention Trainium2 kernel (nn_MultiHeadAttention_7035156430929).

B=4, S=1024, E=1024, H=16, D=64. Sharding: 8 cores = 4 batches x 2
query-halves. Each core computes all 16 heads for its 512 queries against
all 1024 keys of its batch (K/V projections recomputed per pair -> no
collectives). Core outputs are disjoint row-slabs of the (4, 1024, 1024)
output, gathered host-side.

Host prep: activations and weights are pre-transposed with numpy; 1/sqrt(E)
is folded into Wq/bq; the value/output biases fold into one effective output
bias (softmax rows sum to 1); the key-padding mask becomes an additive
per-key bias (-50 -> exp==2e-22) applied inside the exp activation.

Device layouts (per core):
  QT (feat,q), KT (feat,k): projection outputs feature-major (PSUM evict
     on ScalarE with per-partition bias).
  V  (k, feat) token-major, stored head-strided with a ones column per head
     so the PV matmul emits numerator rows 0-63 and the softmax denominator
     in row 64 of one PSUM accumulation.
  PT (k, q) = exp(energy^T + mask_bias_k) per head, straight from PSUM.
  attn^T (feat, q) normalized by reciprocal-denominator broadcast
     (gpsimd partition_broadcast) + vector multiply.
  out (q, e) = attn^T.T @ Wo^T + bias_eff.
All matmuls run as float32r (full PE rate, ~1e-4 rel err on HW).
"""
import sys

sys.path.insert(0, "/opt/trn_rl_repo")

from contextlib import ExitStack

import numpy as np

import concourse.bacc as bacc
import concourse.tile as tile
from concourse import mybir
from concourse.bass_utils import run_bass_kernel_spmd

B, S, E, H, D = 4, 1024, 1024, 16, 64
P = 128
Q = 512            # queries per core
N_CORES = 8
ET = E // P        # 8 emb tiles
NKT = S // P       # 8 key-token tiles
DP1 = D + 1        # V columns per head incl. ones column
F32 = mybir.dt.float32
F32R = mybir.dt.float32r
AF = mybir.ActivationFunctionType


def _declare(nc):
    dp = nc.declare_dram_parameter
    t = {}
    t["xqT"] = dp("xqT", [E, Q], F32R, isOutput=False)
    t["xkT"] = dp("xkT", [E, S], F32R, isOutput=False)
    t["xvT"] = dp("xvT", [E, S], F32R, isOutput=False)
    t["wqT"] = dp("wqT", [E, E], F32R, isOutput=False)  # (emb, feat) prescaled
    t["wkT"] = dp("wkT", [E, E], F32R, isOutput=False)
    t["wvT"] = dp("wvT", [E, E], F32R, isOutput=False)
    t["woT"] = dp("woT", [E, E], F32R, isOutput=False)  # (feat, e_out)
    t["bq"] = dp("bq", [P, ET], F32, isOutput=False)   # col t = f-tile t bias
    t["bk"] = dp("bk", [P, ET], F32, isOutput=False)
    t["maskb"] = dp("maskb", [P, NKT], F32, isOutput=False)  # 0 or -50
    t["boeff"] = dp("boeff", [P, E], F32, isOutput=False)    # row-replicated
    t["ones"] = dp("ones", [P, H], F32R, isOutput=False)
    t["out"] = dp("out", [Q, E], F32, isOutput=True)
    return t


def _emit_body(nc, tc, t):
    xqT, xkT, xvT = t["xqT"], t["xkT"], t["xvT"]
    wqT, wkT, wvT, woT = t["wqT"], t["wkT"], t["wvT"], t["woT"]
    bq, bk, maskb, boeff = t["bq"], t["bk"], t["maskb"], t["boeff"]
    ones, out = t["ones"], t["out"]
    if True:
        ctx = ExitStack()
        const = ctx.enter_context(tc.tile_pool(name="const", bufs=1))
        bq_t = const.tile([P, ET], F32, tag="bq", name="bq")
        bk_t = const.tile([P, ET], F32, tag="bk", name="bk")
        mb_t = const.tile([P, NKT], F32, tag="mb", name="mb")
        bo_t = const.tile([P, E], F32, tag="bo", name="bo")
        nc.gpsimd.dma_start(bq_t[:], bq.ap()[:])
        nc.gpsimd.dma_start(bk_t[:], bk.ap()[:])
        nc.gpsimd.dma_start(mb_t[:], maskb.ap()[:])
        nc.gpsimd.dma_start(bo_t[:], boeff.ap()[:])

        # Persistent per-core intermediates
        qt_pool = ctx.enter_context(tc.tile_pool(name="qt", bufs=ET))
        kt_pool = ctx.enter_context(tc.tile_pool(name="kt", bufs=ET))
        va_pool = ctx.enter_context(tc.tile_pool(name="va", bufs=NKT))
        at_pool = ctx.enter_context(tc.tile_pool(name="at", bufs=2))
        atn_pool = ctx.enter_context(tc.tile_pool(name="atn", bufs=ET))
        den_pool = ctx.enter_context(tc.tile_pool(name="den", bufs=4))
        nrm_pool = ctx.enter_context(tc.tile_pool(name="nrm", bufs=1))
        pt_pool = ctx.enter_context(tc.tile_pool(name="pt", bufs=18))
        wst_pool = ctx.enter_context(tc.tile_pool(name="wst", bufs=10))
        psum = ctx.enter_context(tc.tile_pool(name="psum", bufs=7, space="PSUM"))

        QTs = [qt_pool.tile([P, Q], F32R, tag="qt", name="qt") for _ in range(ET)]
        KTs = [kt_pool.tile([P, S], F32R, tag="kt", name="kt") for _ in range(ET)]
        VAs = [va_pool.tile([P, H * DP1], F32R, tag="va", name="va")
               for _ in range(NKT)]
        ATNs = [atn_pool.tile([P, Q], F32R, tag="atn", name="atn")
                for _ in range(ET)]

        # ---- Q projection: QT[ft] = (Wq_s @ xq.T)[128 feat, 512 q] ----
        with ExitStack() as ph:
            xq_p = ph.enter_context(tc.tile_pool(name="xq", bufs=ET))
            xq_em = [xq_p.tile([P, Q], F32R, tag="xq", name="xq")
                     for _ in range(ET)]
            for et in range(ET):
                nc.sync.dma_start(xq_em[et][:], xqT.ap()[et * P:(et + 1) * P, :])
            for fh in range(2):
                wq_h = [wst_pool.tile([P, Q], F32R, tag="w", name="w")
                        for _ in range(ET)]
                for et in range(ET):
                    nc.sync.dma_start(
                        wq_h[et][:],
                        wqT.ap()[et * P:(et + 1) * P, fh * Q:(fh + 1) * Q])
                for f4 in range(4):
                    ft = fh * 4 + f4
                    ps = psum.tile([P, Q], F32, tag="ps", name="ps")
                    for et in range(ET):
                        nc.tensor.matmul(
                            ps[:], wq_h[et][:, f4 * P:(f4 + 1) * P],
                            xq_em[et][:],
                            start=(et == 0), stop=(et == ET - 1))
                    nc.vector.tensor_scalar_add(QTs[ft][:], ps[:],
                                                bq_t[:, ft:ft + 1])

        # ---- K projection: KT[ft] = (Wk @ xk.T)[128 feat, 1024 k] ----
        with ExitStack() as ph:
            xk_p = ph.enter_context(tc.tile_pool(name="xk", bufs=ET))
            xk_em = [xk_p.tile([P, S], F32R, tag="xk", name="xk")
                     for _ in range(ET)]
            for et in range(ET):
                nc.sync.dma_start(xk_em[et][:], xkT.ap()[et * P:(et + 1) * P, :])
            for fh in range(2):
                wk_h = [wst_pool.tile([P, Q], F32R, tag="w", name="w")
                        for _ in range(ET)]
                for et in range(ET):
                    nc.sync.dma_start(
                        wk_h[et][:],
                        wkT.ap()[et * P:(et + 1) * P, fh * Q:(fh + 1) * Q])
                for f4 in range(4):
                    ft = fh * 4 + f4
                    pss = [psum.tile([P, Q], F32, tag="ps", name="ps")
                           for _ in range(2)]
                    for et in range(ET):
                        for tb in range(2):
                            nc.tensor.matmul(
                                pss[tb][:], wk_h[et][:, f4 * P:(f4 + 1) * P],
                                xk_em[et][:, tb * Q:(tb + 1) * Q],
                                start=(et == 0), stop=(et == ET - 1))
                    for tb in range(2):
                        nc.vector.tensor_scalar_add(
                            KTs[ft][:, tb * Q:(tb + 1) * Q], pss[tb][:],
                            bk_t[:, ft:ft + 1])

        # ---- V projection: V natural (128 tok, feat), head-strided + ones ----
        with ExitStack() as ph:
            xv_p = ph.enter_context(tc.tile_pool(name="xv", bufs=ET))
            xv_em = [xv_p.tile([P, S], F32R, tag="xv", name="xv")
                     for _ in range(ET)]
            for et in range(ET):
                nc.gpsimd.dma_start(xv_em[et][:], xvT.ap()[et * P:(et + 1) * P, :])
            for tt in range(NKT):
                ones3 = VAs[tt][:].rearrange("p (h c) -> p h c", c=DP1)[:, :, D:DP1]
                nc.gpsimd.dma_start(
                    ones3, ones.ap()[:].rearrange("p (h c) -> p h c", c=1))
            for fb in range(2):
                wv_h = [wst_pool.tile([P, Q], F32R, tag="w", name="wv")
                        for _ in range(ET)]
                for et in range(ET):
                    nc.sync.dma_start(
                        wv_h[et][:],
                        wvT.ap()[et * P:(et + 1) * P, fb * Q:(fb + 1) * Q])
                for tt in range(NKT):
                    ps = psum.tile([P, Q], F32, tag="ps", name="ps")
                    for et in range(ET):
                        nc.tensor.matmul(
                            ps[:], xv_em[et][:, tt * P:(tt + 1) * P],
                            wv_h[et][:],
                            start=(et == 0), stop=(et == ET - 1))
                    va3 = VAs[tt][:, fb * 8 * DP1:(fb + 1) * 8 * DP1]
                    va3 = va3.rearrange("p (h c) -> p h c", c=DP1)[:, :, 0:D]
                    ps3 = ps[:].rearrange("p (h c) -> p h c", c=D)
                    nc.vector.tensor_copy(va3, ps3)

        # ---- Attention: per head energy^T -> exp -> PV -> normalize ----
        for j in range(ET):
            at_t = at_pool.tile([P, Q], F32, tag="at", name="at")
            dr = [den_pool.tile([1, Q], F32, tag="drow", name="drow")
                  for _ in range(2)]
            # energy for both heads of the pair interleaved: the two K=64
            # matmuls sit on disjoint PE row-groups (base partition 0 / 64)
            # and execute concurrently on hardware.
            pts = [[], []]
            for kt in range(NKT):
                for hh in range(2):
                    hp = hh * D
                    ps_e = psum.tile([P, Q], F32, tag="ps", name="ps")
                    nc.tensor.matmul(
                        ps_e[:],
                        KTs[j][hp:hp + D, kt * P:(kt + 1) * P],
                        QTs[j][hp:hp + D, :],
                        start=True, stop=True, tile_position=(hp, 0))
                    pt = pt_pool.tile([P, Q], F32R, tag="pt", name="pt")
                    nc.scalar.activation(pt[:], ps_e[:], AF.Exp,
                                         bias=mb_t[:, kt:kt + 1])
                    pts[hh].append(pt)
            for hh in range(2):
                h = 2 * j + hh
                hp = hh * D
                ps_pv = psum.tile([DP1, Q], F32, tag="ps", name="ps")
                for kt in range(NKT):
                    nc.tensor.matmul(
                        ps_pv[:],
                        VAs[kt][:, h * DP1:(h + 1) * DP1],
                        pts[hh][kt][:],
                        start=(kt == 0), stop=(kt == NKT - 1))
                nc.vector.tensor_copy(at_t[hp:hp + D, :], ps_pv[0:D, :])
                nc.vector.tensor_copy(dr[hh][0:1, :], ps_pv[D:DP1, :])
            bc = nrm_pool.tile([P, Q], F32, tag="bc", name="bc")
            tmp = nrm_pool.tile([D, Q], F32, tag="tmp", name="tmp")
            nc.gpsimd.partition_broadcast(bc[0:D, :], dr[0][0:1, :])
            nc.gpsimd.partition_broadcast(tmp[0:D, :], dr[1][0:1, :])
            nc.gpsimd.dma_start(bc[D:P, :], tmp[0:D, :])
            rec = nrm_pool.tile([P, Q], F32, tag="rec", name="rec")
            nc.vector.reciprocal(rec[:], bc[:])
            nc.vector.tensor_mul(ATNs[j][:], at_t[:], rec[:])

        # ---- Output projection: out = attn^T.T @ Wo^T + bias_eff ----
        with ExitStack() as ph:
            ob_p = ph.enter_context(tc.tile_pool(name="ob", bufs=4))
            for eb in range(2):
                pss = [psum.tile([P, Q], F32, tag="ps", name="ps")
                       for _ in range(Q // P)]
                for ft in range(ET):
                    w = wst_pool.tile([P, Q], F32R, tag="w", name="wo")
                    nc.sync.dma_start(
                        w[:], woT.ap()[ft * P:(ft + 1) * P, eb * Q:(eb + 1) * Q])
                    for tt in range(Q // P):
                        nc.tensor.matmul(
                            pss[tt][:],
                            ATNs[ft][:, tt * P:(tt + 1) * P],
                            w[:],
                            start=(ft == 0), stop=(ft == ET - 1))
                for tt in range(Q // P):
                    ob = ob_p.tile([P, Q], F32, tag="ob", name="ob")
                    nc.vector.tensor_add(ob[:], pss[tt][:],
                                         bo_t[:, eb * Q:(eb + 1) * Q])
                    nc.gpsimd.dma_start(
                        out.ap()[tt * P:(tt + 1) * P, eb * Q:(eb + 1) * Q],
                        ob[:])
        ctx.close()


def build_nc(repeats=1, hw_loop=0):
    nc = bacc.Bacc()
    t = _declare(nc)
    with tile.TileContext(nc) as tc:
        if hw_loop:
            with tc.For_i(0, hw_loop, 1):
                _emit_body(nc, tc, t)
        else:
            for _ in range(repeats):
                _emit_body(nc, tc, t)
    nc.finalize()
    return nc


_NC = None


def _get_nc():
    global _NC
    if _NC is None:
        _NC = build_nc()
    return _NC


def _prep_in_maps(value, key_in, query, mask, Wq, bq, Wk, bk, Wv, bv, Wo, bo):
    f = np.float32
    value = np.asarray(value, f)
    key_in = np.asarray(key_in, f)
    query = np.asarray(query, f)
    mask = np.asarray(mask)
    Wq = np.asarray(Wq, f); bq = np.asarray(bq, f)
    Wk = np.asarray(Wk, f); bk = np.asarray(bk, f)
    Wv = np.asarray(Wv, f); bv = np.asarray(bv, f)
    Wo = np.asarray(Wo, f); bo = np.asarray(bo, f)

    s = f(1.0 / np.sqrt(E))
    wqT = np.ascontiguousarray(Wq.T) * s
    wkT = np.ascontiguousarray(Wk.T)
    wvT = np.ascontiguousarray(Wv.T)
    woT = np.ascontiguousarray(Wo.T)
    bq_c = np.ascontiguousarray((bq * s).reshape(ET, P).T)
    bk_c = np.ascontiguousarray(bk.reshape(ET, P).T)
    bo_eff = bo + Wo @ bv
    bo_c = np.ascontiguousarray(np.broadcast_to(bo_eff, (P, E)))

    in_maps = []
    for c in range(N_CORES):
        b, half = c // 2, c % 2
        mrow = mask[b, 0, 0, :]
        mb = np.where(mrow == 0, f(-50.0), f(0.0)).astype(f)
        in_maps.append({
            "xqT": np.ascontiguousarray(query[b, half * Q:(half + 1) * Q, :].T),
            "xkT": np.ascontiguousarray(key_in[b].T),
            "xvT": np.ascontiguousarray(value[b].T),
            "wqT": wqT, "wkT": wkT, "wvT": wvT, "woT": woT,
            "bq": bq_c, "bk": bk_c,
            "maskb": np.ascontiguousarray(mb.reshape(NKT, P).T),
            "boeff": bo_c,
            "ones": np.ones((P, H), np.float32),
        })
    return in_maps


def _assemble(results):
    out = np.empty((B, S, E), np.float32)
    for c in range(N_CORES):
        b, half = c // 2, c % 2
        out[b, half * Q:(half + 1) * Q, :] = results[c]["out"]
    return out


def kernel(value, key_in, query, mask, Wq, bq, Wk, bk, Wv, bv, Wo, bo):
    nc = _get_nc()
    in_maps = _prep_in_maps(value, key_in, query, mask,
                            Wq, bq, Wk, bk, Wv, bv, Wo, bo)
    r = run_bass_kernel_spmd(nc, in_maps, list(range(N_CORES)))
    return _assemble(r.results)


def kernel_traced(value, key_in, query, mask, Wq, bq, Wk, bk, Wv, bv, Wo, bo,
                  **trace_kwargs):
    """Like kernel() but returns (output, BassKernelResults) with profiling."""
    nc = _get_nc()
    in_maps = _prep_in_maps(value, key_in, query, mask,
                            Wq, bq, Wk, bk, Wv, bv, Wo, bo)
    r = run_bass_kernel_spmd(nc, in_maps, list(range(N_CORES)), trace=True,
                             **trace_kwargs)
    return _assemble(r.results), r



# revision 34
# speedup vs baseline: 1.3847x; 1.3847x over previous
"""Multi-head attention Trainium2 kernel (nn_MultiHeadAttention_7035156430929).

B=4, S=1024, E=1024, H=16, D=64. Sharding: 8 cores = 4 batches x 2
head-groups (tensor parallel). Each core computes 8 heads for all 1024
queries of its batch: Wq/Wk/Wv column-sharded by head, Wo row-sharded.
Each core emits a partial (1024, 1024) output (its head-group's slice of
the fc_out contraction); the host sums the two partials per batch (the
unshard step for row-sharded Wo).

Key compaction: masked keys contribute exp(-1e20)=0 to both the softmax
numerator and denominator, so the host gathers only the valid key columns
(per-batch counts ~520/1024) and pads to a multiple of 128. Padded keys
get an additive -50 bias inside the exp (exp==2e-22) and zero V rows.
This cuts K/V projection, energy, exp and PV work to ~5/8.

All matmul operands are bf16 (1 cycle/row on the PE at any moving size;
fp32 PSUM accumulation). Host prep folds 1/sqrt(E) into Wq/bq, the
value/output biases into one effective output bias (softmax rows sum to
1), and the key-padding mask into a per-key additive bias.

Device layouts (per core):
  QT[j] (128 feat, 1024 q), KT[j] (128 feat, C key) bf16: projection
     outputs feature-major, j = head pair 0..3.
  VA (128 tok, kt x 8 heads x 65) bf16 head-strided with a memset ones
     column per head so the PV matmul emits numerator rows 0-63 and the
     softmax denominator in row 64 of one PSUM accumulation.
  PT (128 key, 512 q) bf16 = exp(energy + mask_bias_k) straight from PSUM
     (ScalarE activation with per-partition bias).
  Normalization: DVE reciprocal of each denominator row straight from
     PSUM, gpsimd partition_broadcast across the 64 head partitions, DVE
     multiply (PSUM numerator x SBUF reciprocal) into ATN bf16.
  out (1024, 1024) bf16 partial = ATN.T @ Wo_rows; the bias lands via a
     fifth ones-row matmul in each accumulation group; ScalarE and DVE
     alternate evicting the PSUM groups.

Scheduling: inputs load as one large DMA per tensor (per-DMA ring
overhead ~0.6us), ordered by need with xq/xk split so head pair 0's
energy starts immediately; the exp stream (Activation engine, the
second-longest occupancy after PE) is kept fed by interleaving V/PV/next
projections between the per-pair energy blocks.
"""
import sys

sys.path.insert(0, "/opt/trn_rl_repo")

from contextlib import ExitStack

import numpy as np

import concourse.bacc as bacc
import concourse.tile as tile
from concourse import mybir
from concourse.bass_utils import run_bass_kernel_spmd

B, S, E, H, D = 4, 1024, 1024, 16, 64
P = 128
N_CORES = 8
HL = H // 2          # 8 local heads per core
NJ = HL // 2         # 4 local head pairs
FL = HL * D          # 512 local features
ET = E // P          # 8 emb tiles
FT = FL // P         # 4 local feature tiles
DP1 = D + 1          # V columns per head incl. ones column
F32 = mybir.dt.float32
F32R = mybir.dt.float32r
BF16 = mybir.dt.bfloat16
FP8 = mybir.dt.float8e4
AF = mybir.ActivationFunctionType

try:
    import ml_dtypes
    BF16_NP = ml_dtypes.bfloat16
except ImportError:  # pragma: no cover
    BF16_NP = mybir.dt.np(BF16)
FP8_NP = mybir.dt.np(FP8)


def _declare(nc, nkt):
    C = nkt * P
    dp = nc.declare_dram_parameter
    t = {}
    t["xqT"] = dp("xqT", [E, S], BF16, isOutput=False)   # (emb, q)
    t["xkT"] = dp("xkT", [E, C], BF16, isOutput=False)   # (emb, key) compacted
    t["xvT"] = dp("xvT", [E, C], BF16, isOutput=False)
    t["wqT"] = dp("wqT", [E, FL], BF16, isOutput=False)  # (emb, local feat)
    t["wkT"] = dp("wkT", [E, FL], BF16, isOutput=False)
    t["wvT"] = dp("wvT", [E, FL], BF16, isOutput=False)
    t["woT"] = dp("woT", [FL, E], BF16, isOutput=False)  # (local feat, e_out)
    # packed consts: [:, 0:FT]=bq, [:, FT:2FT]=bk, [:, 2FT:2FT+nkt]=maskb,
    # row 0 of [:, 2FT+nkt:2FT+nkt+E] = half output bias; rows 0-1 of the
    # last 256 columns hold the head-pair selector and a ones row.
    t["cst"] = dp("cst", [P, 2 * FT + nkt + E + 256], F32, isOutput=False)
    t["out"] = dp("out", [S, E], BF16, isOutput=True)
    return t


def _emit_body(nc, tc, t, nkt):
    C = nkt * P
    OBQ, OBK, OMB, OBO = 0, FT, 2 * FT, 2 * FT + nkt
    xqT, xkT, xvT = t["xqT"], t["xkT"], t["xvT"]
    wqT, wkT, wvT, woT = t["wqT"], t["wkT"], t["wvT"], t["woT"]
    cst, out = t["cst"], t["out"]

    OIS = OBO + E
    ctx = ExitStack()
    const = ctx.enter_context(tc.tile_pool(name="const", bufs=1))
    cst_t = const.tile([P, 2 * FT + nkt + E], F32, tag="cst", name="cst")
    iso_t = const.tile([2, 256], F32R, tag="iso", name="iso")
    bo_t = const.tile([1, E], F32R, tag="bor", name="bor")
    bq_t = cst_t[:, OBQ:OBQ + FT]
    bk_t = cst_t[:, OBK:OBK + FT]
    mb_t = cst_t[:, OMB:OMB + nkt]
    one_t = iso_t[0:1, P:2 * P]

    # Inputs live as one wide SBUF tile per tensor (emb-tile-major
    # columns) so each loads with a single large DMA.
    big = ctx.enter_context(tc.tile_pool(name="big", bufs=1))
    qt_pool = ctx.enter_context(tc.tile_pool(name="qt", bufs=FT))
    kt_pool = ctx.enter_context(tc.tile_pool(name="kt", bufs=FT))
    va_pool = ctx.enter_context(tc.tile_pool(name="va", bufs=1))
    atn_pool = ctx.enter_context(tc.tile_pool(name="atn", bufs=NJ))
    rec_pool = ctx.enter_context(tc.tile_pool(name="rec", bufs=3))
    pt_pool = ctx.enter_context(tc.tile_pool(name="pt", bufs=8 * nkt + 4))
    ob_pool = ctx.enter_context(tc.tile_pool(name="ob", bufs=4))
    psum = ctx.enter_context(tc.tile_pool(name="psum", bufs=5, space="PSUM"))
    psum_pv = ctx.enter_context(tc.tile_pool(name="psumpv", bufs=3,
                                             space="PSUM"))

    # xq/xk split into one tile per first-use chunk: a single wide tile
    # would serialize the later chunk's DMA behind the earlier chunk's
    # readers (tile-granular write-after-read dependency).
    kchunks = []
    koff = 0
    while koff < C:
        kchunks.append((koff, min(512, C - koff)))
        koff += min(512, C - koff)
    xq_sbs = [big.tile([P, ET * 512], BF16, tag=f"xq{qh}", name="xq")
              for qh in range(2)]
    xk_sbs = [big.tile([P, ET * w], BF16, tag=f"xk{i}", name="xk")
              for i, (_, w) in enumerate(kchunks)]
    xv_sb = big.tile([P, ET * C], BF16, tag="xv", name="xv")
    wq_sb = big.tile([P, ET * FL], BF16, tag="wq", name="wq")
    wk_sb = big.tile([P, ET * FL], BF16, tag="wk", name="wk")
    wv_sb = big.tile([P, ET * FL], BF16, tag="wv", name="wv")
    wo_sb = big.tile([P, FT * E], BF16, tag="wo", name="wo")
    va_sb = va_pool.tile([P, nkt * HL * DP1], BF16, tag="va", name="va")
    xq_em = [[xq_sbs[qh][:, et * 512:(et + 1) * 512] for et in range(ET)]
             for qh in range(2)]
    xk_em = [[sb[:, et * w:(et + 1) * w] for et in range(ET)]
             for sb, (_, w) in zip(xk_sbs, kchunks)]
    xv_em = [xv_sb[:, et * C:(et + 1) * C] for et in range(ET)]
    wq_t = [wq_sb[:, et * FL:(et + 1) * FL] for et in range(ET)]
    wk_t = [wk_sb[:, et * FL:(et + 1) * FL] for et in range(ET)]
    wv_t = [wv_sb[:, et * FL:(et + 1) * FL] for et in range(ET)]
    wo_t = [wo_sb[:, ft * E:(ft + 1) * E] for ft in range(FT)]
    VAs = [va_sb[:, kt * HL * DP1:(kt + 1) * HL * DP1] for kt in range(nkt)]
    QTs = [qt_pool.tile([P, S], BF16, tag="qt", name="qt") for _ in range(NJ)]
    KTs = [kt_pool.tile([P, C], BF16, tag="kt", name="kt") for _ in range(NJ)]
    ATNs = [atn_pool.tile([P, S], BF16, tag="atn", name="atn")
            for _ in range(NJ)]

    # VA ones columns built on-device; partial-partition constants
    # (selector, ones row, bias row) DMA in from the packed const tensor.
    va4 = va_sb[:].rearrange("p (t h c) -> p t h c", h=HL, c=DP1)
    nc.gpsimd.memset(va4[:, :, :, D:DP1], 1.0)

    def chunk_dma(eng, sb, w, dram, ncols, off):
        # columns [off, off+w) of every emb tile, as 3D APs
        sb3 = sb[:].rearrange("p (a q) -> p a q", q=w)
        dr3 = dram.rearrange("(a p) q -> p a q", p=P)[:, :, off:off + w]
        eng.dma_start(sb3, dr3)

    # Input DMAs in need order. The DMA engines round-robin the two
    # rings, so only wq/wk (needed just as early as the first activation
    # chunks) go on the sync ring; everything else queues on scalar in
    # exactly first-use order: pair-0's gates, then the rest.
    nc.scalar.dma_start(cst_t[:], cst.ap()[:, 0:2 * FT + nkt + E])
    nc.scalar.dma_start(iso_t[:], cst.ap()[0:2, OIS:OIS + 256].bitcast(F32R))
    nc.scalar.dma_start(bo_t[:], cst.ap()[0:1, OBO:OBO + E].bitcast(F32R))
    nc.sync.dma_start(wq_sb[:].rearrange("p (a q) -> p a q", q=FL),
                      wqT.ap()[:].rearrange("(a p) q -> p a q", p=P))
    chunk_dma(nc.scalar, xq_sbs[0], 512, xqT.ap()[:], S, 0)
    nc.sync.dma_start(wk_sb[:].rearrange("p (a q) -> p a q", q=FL),
                      wkT.ap()[:].rearrange("(a p) q -> p a q", p=P))
    for sb, (off, w) in zip(xk_sbs, kchunks):
        chunk_dma(nc.scalar, sb, w, xkT.ap()[:], C, off)
    chunk_dma(nc.scalar, xq_sbs[1], 512, xqT.ap()[:], S, 512)
    nc.scalar.dma_start(xv_sb[:].rearrange("p (a q) -> p a q", q=C),
                        xvT.ap()[:].rearrange("(a p) q -> p a q", p=P))
    nc.scalar.dma_start(wv_sb[:].rearrange("p (a q) -> p a q", q=FL),
                        wvT.ap()[:].rearrange("(a p) q -> p a q", p=P))
    nc.scalar.dma_start(wo_sb[:].rearrange("p (a q) -> p a q", q=E),
                        woT.ap()[:].rearrange("(a p) q -> p a q", p=P))

    def emit_qproj(j, qhs=(0, 1)):
        for qh in qhs:
            ps = psum.tile([P, 512], F32, tag="ps", name="ps")
            for et in range(ET):
                nc.tensor.matmul(
                    ps[:], wq_t[et][:, j * P:(j + 1) * P],
                    xq_em[qh][et][:],
                    start=(et == 0), stop=(et == ET - 1))
            nc.vector.tensor_scalar_add(
                QTs[j][:, qh * 512:(qh + 1) * 512], ps[:], bq_t[:, j:j + 1])

    def emit_kproj(j):
        for ci, (off, w) in enumerate(kchunks):
            ps = psum.tile([P, 512], F32, tag="ps", name="ps")
            for et in range(ET):
                nc.tensor.matmul(
                    ps[:, 0:w], wk_t[et][:, j * P:(j + 1) * P],
                    xk_em[ci][et][:],
                    start=(et == 0), stop=(et == ET - 1))
            nc.vector.tensor_scalar_add(
                KTs[j][:, off:off + w], ps[:, 0:w], bk_t[:, j:j + 1])

    # pts[j][qh][hh][kt]; pv_norm(j) may lag energy(j) by one pair, so the
    # pt pool holds two pairs' worth of tiles.
    pts = {}

    def emit_energy(j, qhs=(0, 1)):
        if j not in pts:
            pts[j] = [[[None] * nkt for _ in range(2)] for _ in range(2)]
        for qh in qhs:
            for kt in range(nkt):
                for hh in range(2):
                    hp = hh * D
                    ps_e = psum.tile([P, 512], F32, tag="ps", name="ps")
                    nc.tensor.matmul(
                        ps_e[:],
                        KTs[j][hp:hp + D, kt * P:(kt + 1) * P],
                        QTs[j][hp:hp + D, qh * 512:(qh + 1) * 512],
                        start=True, stop=True, tile_position=(hp, 0))
                    pt = pt_pool.tile([P, 512], BF16, tag="pt", name="pt")
                    nc.scalar.activation(pt[:], ps_e[:], AF.Exp,
                                         bias=mb_t[:, kt:kt + 1])
                    pts[j][qh][hh][kt] = pt

    def emit_pv_norm(j, qh):
        ps_pv = [None, None]
        for hh in range(2):
            h = 2 * j + hh
            pv = psum_pv.tile([DP1, 512], F32, tag="pspv", name="pspv")
            ps_pv[hh] = pv
            for kt in range(nkt):
                nc.tensor.matmul(
                    pv[:],
                    VAs[kt][:, h * DP1:(h + 1) * DP1],
                    pts[j][qh][hh][kt][:],
                    start=(kt == 0), stop=(kt == nkt - 1))
        for hh in range(2):
            rec = rec_pool.tile([1, 512], F32R, tag="rec", name="rec")
            with nc.allow_low_precision(reason="f32r reciprocal, same bits"):
                nc.vector.reciprocal(rec[:], ps_pv[hh][D:DP1, :])
            bc = rec_pool.tile([D, 512], F32R, tag="bc", name="bc")
            nc.gpsimd.partition_broadcast(bc[:], rec[:])
            nc.vector.tensor_mul(
                ATNs[j][hh * D:hh * D + D, qh * 512:(qh + 1) * 512],
                ps_pv[hh][0:D, :], bc[:])

    def emit_vproj():
        for kt in range(nkt):
            ps = psum.tile([P, 512], F32, tag="ps", name="ps")
            for et in range(ET):
                nc.tensor.matmul(
                    ps[:], xv_em[et][:, kt * P:(kt + 1) * P],
                    wv_t[et][:],
                    start=(et == 0), stop=(et == ET - 1))
            va3 = VAs[kt][:].rearrange("p (h c) -> p h c", c=DP1)[:, :, 0:D]
            ps3 = ps[:].rearrange("p (h c) -> p h c", c=D)
            nc.vector.tensor_copy(va3, ps3)

    # ---- Output projection: out_partial = ATN.T @ Wo_rows + bias/2 ----
    # Bias rides in as a fifth ones-row matmul; ScalarE and DVE alternate
    # evicting the PSUM groups.
    def emit_outproj(qh):
        for eb in range(2):
            for qc in range(qh * 4, qh * 4 + 4):
                ps_o = psum.tile([P, 512], F32, tag="ps", name="ps")
                for j in range(NJ):
                    nc.tensor.matmul(
                        ps_o[:],
                        ATNs[j][:, qc * P:(qc + 1) * P],
                        wo_t[j][:, eb * 512:(eb + 1) * 512],
                        start=(j == 0), stop=False)
                nc.tensor.matmul(
                    ps_o[:], one_t,
                    bo_t[:, eb * 512:(eb + 1) * 512],
                    start=False, stop=True)
                ob = ob_pool.tile([P, 512], BF16, tag="ob", name="ob")
                if qc % 2 == 0:
                    nc.scalar.activation(ob[:], ps_o[:], AF.Copy)
                else:
                    nc.vector.tensor_copy(ob[:], ps_o[:])
                ring = nc.sync if (qc % 2 == 0) else nc.scalar
                ring.dma_start(
                    out.ap()[qc * P:(qc + 1) * P, eb * 512:(eb + 1) * 512],
                    ob[:])

    # Query-half pipeline: all pairs' qh0 energy/exp first, then the qh0
    # output projection overlaps qh1's exp stream, halving the tail and
    # filling what would otherwise be Activation-engine idle gaps.
    emit_qproj(0, (0,))
    emit_kproj(0)
    emit_energy(0, (0,))
    emit_vproj()
    emit_qproj(1, (0,))
    emit_kproj(1)
    emit_energy(1, (0,))
    emit_pv_norm(0, 0)
    emit_qproj(2, (0,))
    emit_kproj(2)
    emit_energy(2, (0,))
    emit_pv_norm(1, 0)
    emit_qproj(3, (0,))
    emit_kproj(3)
    emit_energy(3, (0,))
    emit_pv_norm(2, 0)
    emit_pv_norm(3, 0)
    emit_qproj(0, (1,))
    emit_energy(0, (1,))
    emit_outproj(0)
    emit_qproj(1, (1,))
    emit_energy(1, (1,))
    emit_pv_norm(0, 1)
    emit_qproj(2, (1,))
    emit_energy(2, (1,))
    emit_pv_norm(1, 1)
    emit_qproj(3, (1,))
    emit_energy(3, (1,))
    emit_pv_norm(2, 1)
    emit_pv_norm(3, 1)
    emit_outproj(1)
    ctx.close()


_NKT = 5  # key tiles; overwritten by _prep_in_maps from the runtime mask


def build_nc(repeats=1, hw_loop=0, nkt=None):
    if nkt is None:
        nkt = _NKT
    nc = bacc.Bacc()
    t = _declare(nc, nkt)
    with tile.TileContext(nc) as tc:
        if hw_loop:
            with tc.For_i(0, hw_loop, 1):
                _emit_body(nc, tc, t, nkt)
        else:
            for _ in range(repeats):
                _emit_body(nc, tc, t, nkt)
    nc.finalize()
    return nc


_NC = {}


def _get_nc(nkt):
    if nkt not in _NC:
        _NC[nkt] = build_nc(nkt=nkt)
    return _NC[nkt]


def _prep_in_maps(value, key_in, query, mask, Wq, bq, Wk, bk, Wv, bv, Wo, bo):
    global _NKT
    f = np.float32
    value = np.asarray(value, f)
    key_in = np.asarray(key_in, f)
    query = np.asarray(query, f)
    mask = np.asarray(mask)
    Wq = np.asarray(Wq, f); bq = np.asarray(bq, f)
    Wk = np.asarray(Wk, f); bk = np.asarray(bk, f)
    Wv = np.asarray(Wv, f); bv = np.asarray(bv, f)
    Wo = np.asarray(Wo, f); bo = np.asarray(bo, f)

    mrows = mask[:, 0, 0, :]
    idxs = [np.nonzero(mrows[b])[0] for b in range(B)]
    maxvalid = max(1, max(len(ix) for ix in idxs))
    nkt = (maxvalid + P - 1) // P
    _NKT = nkt
    C = nkt * P

    s = f(1.0 / np.sqrt(E))
    wqT = np.ascontiguousarray(Wq.T) * s          # (emb, feat)
    wkT = np.ascontiguousarray(Wk.T)
    wvT = np.ascontiguousarray(Wv.T)
    woT = np.ascontiguousarray(Wo.T)              # (feat, e_out)

    in_maps = []
    for c in range(N_CORES):
        b, g = c // 2, c % 2
        fs = slice(g * FL, (g + 1) * FL)
        ix = idxs[b]
        nv = len(ix)

        xk_c = np.zeros((E, C), f)
        xv_c = np.zeros((E, C), f)
        xk_c[:, :nv] = key_in[b].T[:, ix]
        xv_c[:, :nv] = value[b].T[:, ix]
        mb = np.full((C,), f(-50.0))
        mb[:nv] = 0.0

        bo_eff = bo * f(0.5) + Wo[:, fs] @ bv[fs]
        cst = np.zeros((P, 2 * FT + nkt + E + 256), f)
        cst[:, 0:FT] = (bq[fs] * s).reshape(FT, P).T
        cst[:, FT:2 * FT] = bk[fs].reshape(FT, P).T
        cst[:, 2 * FT:2 * FT + nkt] = mb.reshape(nkt, P).T
        obo = 2 * FT + nkt
        cst[0, obo:obo + E] = bo_eff
        cst[0, obo + E:obo + E + D] = 1.0          # selector row 0
        cst[1, obo + E + D:obo + E + P] = 1.0      # selector row 1
        cst[0, obo + E + P:obo + E + 2 * P] = 1.0  # ones row

        in_maps.append({
            "xqT": np.ascontiguousarray(query[b].T).astype(BF16_NP),
            "xkT": xk_c.astype(BF16_NP),
            "xvT": xv_c.astype(BF16_NP),
            "wqT": np.ascontiguousarray(wqT[:, fs]).astype(BF16_NP),
            "wkT": np.ascontiguousarray(wkT[:, fs]).astype(BF16_NP),
            "wvT": np.ascontiguousarray(wvT[:, fs]).astype(BF16_NP),
            "woT": np.ascontiguousarray(woT[fs, :]).astype(BF16_NP),
            "cst": cst,
        })
    return in_maps


def _assemble(results):
    out = np.empty((B, S, E), np.float32)
    for b in range(B):
        out[b] = (results[2 * b]["out"].astype(np.float32)
                  + results[2 * b + 1]["out"].astype(np.float32))
    return out


def kernel(value, key_in, query, mask, Wq, bq, Wk, bk, Wv, bv, Wo, bo):
    in_maps = _prep_in_maps(value, key_in, query, mask,
                            Wq, bq, Wk, bk, Wv, bv, Wo, bo)
    nc = _get_nc(_NKT)
    r = run_bass_kernel_spmd(nc, in_maps, list(range(N_CORES)))
    return _assemble(r.results)


def kernel_traced(value, key_in, query, mask, Wq, bq, Wk, bk, Wv, bv, Wo, bo,
                  **trace_kwargs):
    """Like kernel() but returns (output, BassKernelResults) with profiling."""
    in_maps = _prep_in_maps(value, key_in, query, mask,
                            Wq, bq, Wk, bk, Wv, bv, Wo, bo)
    nc = _get_nc(_NKT)
    r = run_bass_kernel_spmd(nc, in_maps, list(range(N_CORES)), trace=True,
                             **trace_kwargs)
    return _assemble(r.results), r
